# revision 1
# baseline (speedup 1.0000x reference)
"""MoE (top-2 of 8 experts, D=1024) — Trainium2 Bass kernel, expert-parallel on 8 cores.

Strategy: the router (softmax + top-2 over E=8) is tiny and data-dependent, so it
runs on host in fp64. Tokens are dispatched (gathered) per expert on host; core e
receives the tokens routed to expert e (padded to a static capacity C), that
expert's w1/w2, and the per-token gate weights. Each core computes
    y = gate * (gelu(x @ w1) @ w2)
for its token batch using bf16 matmuls with fp32 PSUM accumulation. The host
scatter-adds the K=2 expert contributions per token back to the full output.
"""

import numpy as np
import ml_dtypes
from contextlib import ExitStack

import concourse.bass as bass
import concourse.tile as tile
import concourse.mybir as mybir
from concourse import bacc
from concourse.bass_utils import run_bass_kernel_spmd

E, T, D, K = 8, 32768, 1024, 2
N_CORES = 8
P = 128
TOK_TILE = 512  # tokens per pipeline tile
BF16 = mybir.dt.bfloat16
F32 = mybir.dt.float32
NP_BF16 = ml_dtypes.bfloat16

_nc_cache = {}


def build_nc(C):
    """Build the per-core Bass program for token capacity C (multiple of TOK_TILE)."""
    assert C % TOK_TILE == 0
    KC = D // P        # contraction chunks (8)
    NT = C // TOK_TILE  # token tiles

    nc = bacc.Bacc("TRN2", target_bir_lowering=False, debug=False)
    xT = nc.dram_tensor("xT", [D, C], BF16, kind="ExternalInput").ap()
    w1 = nc.dram_tensor("w1", [D, D], BF16, kind="ExternalInput").ap()
    w2 = nc.dram_tensor("w2", [D, D], BF16, kind="ExternalInput").ap()
    gT = nc.dram_tensor("gT", [P, C // P], F32, kind="ExternalInput").ap()
    y = nc.dram_tensor("y", [C, D], F32, kind="ExternalOutput").ap()

    with tile.TileContext(nc) as tc, ExitStack() as ctx:
        const = ctx.enter_context(tc.tile_pool(name="const", bufs=1))
        w1_sb = const.tile([P, KC, D], BF16)
        w2_sb = const.tile([P, KC, D], BF16)
        g_sb = const.tile([P, C // P], F32)
        for k in range(KC):
            nc.sync.dma_start(w1_sb[:, k, :], w1[k * P:(k + 1) * P, :])
            nc.sync.dma_start(w2_sb[:, k, :], w2[k * P:(k + 1) * P, :])
        nc.sync.dma_start(g_sb[:], gT[:])

        x_pool = ctx.enter_context(tc.tile_pool(name="x", bufs=3))
        h_pool = ctx.enter_context(tc.tile_pool(name="h", bufs=2))
        y_pool = ctx.enter_context(tc.tile_pool(name="yo", bufs=4))
        ps1 = ctx.enter_context(tc.tile_pool(name="ps1", bufs=4, space="PSUM"))
        ps2 = ctx.enter_context(tc.tile_pool(name="ps2", bufs=4, space="PSUM"))

        for m in range(NT):
            # xT tile for TOK_TILE tokens: [k-chunk partitions, chunk, token]
            x_sb = x_pool.tile([P, KC, TOK_TILE], BF16)
            for k in range(KC):
                nc.sync.dma_start(
                    x_sb[:, k, :], xT[k * P:(k + 1) * P, bass.ts(m, TOK_TILE)]
                )
            # Layer 1: hT[n*P:(n+1)*P, tokens] = (w1 chunk).T @ xT chunk, + gelu
            h_sb = h_pool.tile([P, KC, TOK_TILE], BF16)
            for n in range(KC):
                hp = ps1.tile([P, TOK_TILE], F32)
                for k in range(KC):
                    nc.tensor.matmul(
                        hp[:],
                        w1_sb[:, k, n * P:(n + 1) * P],
                        x_sb[:, k, :],
                        start=(k == 0),
                        stop=(k == KC - 1),
                    )
                nc.scalar.activation(
                    h_sb[:, n, :], hp[:], mybir.ActivationFunctionType.Gelu
                )
            # Layer 2: y[tokens, :] = gate * (h @ w2), token sub-tiles of 128
            for t in range(TOK_TILE // P):
                j = m * (TOK_TILE // P) + t  # global 128-token chunk index
                for n2 in range(2):
                    yp = ps2.tile([P, 512], F32)
                    for k in range(KC):
                        nc.tensor.matmul(
                            yp[:],
                            h_sb[:, k, t * P:(t + 1) * P],
                            w2_sb[:, k, n2 * 512:(n2 + 1) * 512],
                            start=(k == 0),
                            stop=(k == KC - 1),
                        )
                    y_sb = y_pool.tile([P, 512], F32)
                    nc.vector.tensor_scalar_mul(y_sb[:], yp[:], g_sb[:, j:j + 1])
                    nc.sync.dma_start(
                        y[j * P:(j + 1) * P, n2 * 512:(n2 + 1) * 512], y_sb[:]
                    )
    nc.compile()
    return nc


def _get_nc(C):
    if C not in _nc_cache:
        _nc_cache[C] = build_nc(C)
    return _nc_cache[C]


def route_and_dispatch(tokens, router_w):
    """Host router: fp64 softmax + top-2. Returns per-expert index/gate arrays."""
    logits = tokens.astype(np.float64) @ router_w.astype(np.float64).T  # [T, E]
    logits -= logits.max(axis=-1, keepdims=True)
    p = np.exp(logits)
    p /= p.sum(axis=-1, keepdims=True)
    t_ar = np.arange(tokens.shape[0])
    i0 = p.argmax(-1)
    v0 = p[t_ar, i0]
    p[t_ar, i0] = -1.0
    i1 = p.argmax(-1)
    v1 = p[t_ar, i1]
    p[t_ar, i0] = v0  # restore
    idx, gates = [], []
    for e in range(E):
        sel0 = i0 == e
        sel1 = i1 == e
        ids = np.nonzero(sel0 | sel1)[0]
        g = np.where(sel0[ids], v0[ids], 0.0) + np.where(sel1[ids], v1[ids], 0.0)
        idx.append(ids)
        gates.append(g.astype(np.float32))
    return idx, gates


def kernel(tokens, router_w, w1, w2):
    tokens = np.asarray(tokens)
    router_w = np.asarray(router_w)
    w1 = np.asarray(w1)
    w2 = np.asarray(w2)
    T_, D_ = tokens.shape
    assert D_ == D and router_w.shape == (E, D)

    idx, gates = route_and_dispatch(tokens, router_w)
    max_n = max(len(i) for i in idx)
    C = max(TOK_TILE, ((max_n + TOK_TILE - 1) // TOK_TILE) * TOK_TILE)

    tokT = np.ascontiguousarray(tokens.astype(NP_BF16).T)  # [D, T] bf16
    in_maps = []
    for e in range(E):
        n_e = len(idx[e])
        xT = np.zeros((D, C), NP_BF16)
        xT[:, :n_e] = tokT[:, idx[e]]
        g = np.zeros(C, np.float32)
        g[:n_e] = gates[e]
        gT = np.ascontiguousarray(g.reshape(C // P, P).T)  # [P, C//P]
        in_maps.append(
            {
                "xT": xT,
                "w1": np.ascontiguousarray(w1[e]).astype(NP_BF16),
                "w2": np.ascontiguousarray(w2[e]).astype(NP_BF16),
                "gT": gT,
            }
        )

    nc = _get_nc(C)
    res = run_bass_kernel_spmd(nc, in_maps, core_ids=list(range(N_CORES)))

    out = np.zeros((T_, D), np.float32)
    for e in range(E):
        n_e = len(idx[e])
        if n_e:
            out[idx[e]] += res.results[e]["y"][:n_e]
    return out
